# revision 35
# baseline (speedup 1.0000x reference)
"""Distributed causal attention with RoPE for trn2 (8 NeuronCores).

Problem: B=2, S=2048, DIM=2048, H=16 heads, D=128.
  out = softmax(causal(rope(xq) @ rope(xk)^T / sqrt(D))) @ xv @ wo^T

Sharding: tensor-parallel over heads, 8-way: each core owns 2 global heads
for BOTH batches (4 local attention instances).  Attention is fully local;
the only collective is an 8-rank AllToAll per sequence chunk, which leaves
every core with the full per-batch attention output in global-head-major
row order (identical static indices on every core -> clean SPMD).  Each
core then computes one (batch, 512-column) slice of the output projection.

The whole kernel is one software pipeline over the 4 sequence chunks:
  proj(chunk t) -> attention(t) -> AllToAll(t) -> out-projection(t-2)
so the collective latency hides behind the dense projection/attention
matmul stream.

Layout strategy (on-device matmuls contract over the partition axis):
  - host feeds x^T and w^T so no on-device transposes are needed
  - Q,K are produced transposed ([d, s]); RoPE pair-swap runs on the
    vector engine via stream_shuffle straight out of PSUM
  - softmax runs on transposed scores, flash-style streamed over k-tiles:
    exp on ACT, causal masking by multiply, then the k-tile's contribution
    is immediately accumulated into both the PV product and the row-sum
    (a ones-matrix matmul whose output is replicated across partitions,
    so the broadcast for the final normalization is free)
  - compute dtype bf16 (fp32 PSUM accumulation), output fp32
"""

import math
import sys

sys.path.insert(0, "/opt/trn_rl_repo")

import numpy as np
import ml_dtypes

import concourse.bass as bass
import concourse.mybir as mybir
import concourse.tile as tile
from concourse import bacc
from concourse.bass_utils import run_bass_kernel_spmd

BF16 = mybir.dt.bfloat16
F32 = mybir.dt.float32

B, S, DIM, H, D = 2, 2048, 2048, 16, 128
NCORES = 8
HPC = 2                  # global heads per core
NI = B * HPC             # local attention instances (batch x head) = 4
HD = HPC * D             # local hidden slice = 256
OSL = 512                # output column slice per core
GROUP = [list(range(NCORES))]
INV_SQRT_D = 1.0 / math.sqrt(D)
SWAP_MASK = [i + 1 if i % 2 == 0 else i - 1 for i in range(32)]

SBW = 512                # s-block width (= pipeline chunk width)
NSB = S // SBW           # 4
NKT = DIM // 128         # 16 contraction tiles for projections

LAST_RESULT = None
_CACHED_NC = None


def _proj_block(nc, b, sb, xt, wq_t, wk_t, wv_t, cos_t, sin_t,
                qTc, kT_t, v_t, ta, psP):
    """QKV projections (+RoPE on Q,K) for (batch b, s-block sb).

    Q^T goes to the per-chunk tile qTc [128, NI, 512]; K^T to the resident
    kT_t at the chunk's s-range.
    """
    ssl = slice(sb * SBW, (sb + 1) * SBW)
    for wi, (w_t, qk) in enumerate(((wq_t, 0), (wk_t, 1))):
        for j in range(HPC):
            hi = b * HPC + j
            pp = psP.tile([128, SBW], F32, tag="acc", bufs=4,
                          name=f"pp{b}{sb}{j}{wi}")
            for i in range(NKT):
                nc.tensor.matmul(
                    pp[:],
                    lhsT=w_t[:, i, j * 128:(j + 1) * 128],
                    rhs=xt[:, i, :],
                    start=(i == 0), stop=(i == NKT - 1))
            # rope: out = x*cos + pair_swap(x)*sin_signed
            dst = qTc[:, hi, :] if qk == 0 else kT_t[:, hi, ssl]
            swq = ta.tile([128, SBW], F32, tag="swq", bufs=2,
                          name=f"swq{b}{sb}{j}{wi}")
            nc.vector.stream_shuffle(swq[:], pp[:], SWAP_MASK)
            nc.vector.tensor_tensor(
                out=dst, in0=pp[:], in1=cos_t[:, ssl],
                op=mybir.AluOpType.mult)
            m2 = ta.tile([128, SBW], BF16, tag="m2", bufs=2,
                         name=f"m2_{b}{sb}{j}{wi}")
            nc.vector.tensor_tensor(
                out=m2[:], in0=swq[:], in1=sin_t[:, ssl],
                op=mybir.AluOpType.mult)
            nc.vector.tensor_tensor(
                out=dst, in0=dst, in1=m2[:], op=mybir.AluOpType.add)
    # V in natural layout [s, dv]
    for m in range(4):
        pv = psP.tile([128, HD], F32, tag="acc", bufs=4, name=f"pv{b}{sb}{m}")
        for i in range(NKT):
            nc.tensor.matmul(
                pv[:],
                lhsT=xt[:, i, m * 128:(m + 1) * 128],
                rhs=wv_t[:, i, :],
                start=(i == 0), stop=(i == NKT - 1))
        nc.scalar.copy(v_t[:, b * 16 + 4 * sb + m, :], pv[:])


def _att_block(nc, qb, hi, qTc, kT_t, v_t, ao_t, msk_t, ones_m, eb, tb, psP):
    """Flash-style attention for one (q-block, instance).

    Streams k-tiles: scores -> exp -> mask, then immediately accumulates
    the row-sum (pr) and PV (pu) contributions so only a few E tiles are
    ever live.  Emission staggers the consumers one k-tile behind the
    scores matmul to hide the ACT exp latency.
    """
    b, j = hi // HPC, hi % HPC
    nkt = 4 * qb + 4
    pr = psP.tile([128, 512], F32, tag="r", bufs=2, name=f"pr{qb}{hi}")
    pu = psP.tile([128, 512], F32, tag="acc", bufs=4, name=f"pu{qb}{hi}")

    def scores(kt):
        dj = kt - 4 * qb
        c0 = max(dj, 0) * 128  # diagonal tiles: columns < c0 fully masked
        pk = psP.tile([128, 512], F32, tag="sc", bufs=2,
                      name=f"pk{qb}{hi}{kt}")
        nc.tensor.matmul(
            pk[:, c0:],
            lhsT=kT_t[:, hi, kt * 128:(kt + 1) * 128],
            rhs=qTc[:, hi, c0:],
            start=True, stop=True)
        et = eb.tile([128, 512], BF16, tag="e", bufs=4, name=f"et{qb}{hi}{kt}")
        nc.scalar.activation(
            et[:, c0:], pk[:, c0:], mybir.ActivationFunctionType.Exp,
            scale=INV_SQRT_D)
        if dj >= 0:
            nc.vector.tensor_tensor(
                out=et[:, c0:], in0=et[:, c0:], in1=msk_t[:, dj, c0:],
                op=mybir.AluOpType.mult)
        return et, c0

    def reduce(kt, et, c0):
        nc.tensor.matmul(pr[:, c0:], lhsT=ones_m[:], rhs=et[:, c0:],
                         start=(kt == 0), stop=(kt == nkt - 1))
        nc.tensor.matmul(
            pu[:, c0:],
            lhsT=v_t[:, b * 16 + kt, j * 128:(j + 1) * 128],
            rhs=et[:, c0:],
            start=(kt == 0), stop=(kt == nkt - 1))

    pend = []
    for kt in range(nkt):
        pend.append((kt, scores(kt)))
        if len(pend) > 2:
            k0, (et0, c00) = pend.pop(0)
            reduce(k0, et0, c00)
    for k0, (et0, c00) in pend:
        reduce(k0, et0, c00)

    rinv = tb.tile([128, 512], F32, tag="rinv", bufs=2, name=f"ri{qb}{hi}")
    nc.vector.reciprocal(rinv[:], pr[:])
    nc.vector.tensor_tensor(
        out=ao_t[:, hi, :], in0=pu[:], in1=rinv[:],
        op=mybir.AluOpType.mult)


def _out_proj_block(nc, qb, agt_ab, wo_t, out, tco, psP):
    """Output projection for s-chunk qb from the AllToAll-delivered heads."""
    agt_a, agt_b = agt_ab
    for st in range(4):
        po = psP.tile([128, OSL], F32, tag="acc", bufs=4, name=f"po{qb}{st}")
        for i in range(8):
            nc.tensor.matmul(
                po[:],
                lhsT=agt_a[:, i, st * 128:(st + 1) * 128],
                rhs=wo_t[:, i, :],
                start=(i == 0), stop=False)
        for i in range(8):
            nc.tensor.matmul(
                po[:],
                lhsT=agt_b[:, i, st * 128:(st + 1) * 128],
                rhs=wo_t[:, i + 8, :],
                start=False, stop=(i == 7))
        ot = tco.tile([128, OSL], F32, tag="ot", bufs=2, name=f"ot{qb}{st}")
        nc.vector.tensor_copy(ot[:], po[:])
        r0 = qb * 512 + st * 128
        nc.sync.dma_start(out=out[r0:r0 + 128, :], in_=ot[:])


def _build():
    nc = bacc.Bacc("TRN2", target_bir_lowering=False, debug=False,
                   num_devices=NCORES)

    xT0 = nc.declare_dram_parameter("xT0", [DIM, S], BF16, isOutput=False)
    xT1 = nc.declare_dram_parameter("xT1", [DIM, S], BF16, isOutput=False)
    wqT = nc.declare_dram_parameter("wqT", [DIM, HD], BF16, isOutput=False)
    wkT = nc.declare_dram_parameter("wkT", [DIM, HD], BF16, isOutput=False)
    wvT = nc.declare_dram_parameter("wvT", [DIM, HD], BF16, isOutput=False)
    woT = nc.declare_dram_parameter("woT", [DIM, OSL], BF16, isOutput=False)
    cosb = nc.declare_dram_parameter("cosb", [128, S], BF16, isOutput=False)
    sinb = nc.declare_dram_parameter("sinb", [128, S], BF16, isOutput=False)
    msk = nc.declare_dram_parameter("msk", [4, 128, 512], BF16, isOutput=False)
    out = nc.declare_dram_parameter("out", [S, OSL], F32, isOutput=True)
    xTs = (xT0, xT1)

    with tile.TileContext(nc) as tc:
        with (
            tc.tile_pool(name="res", bufs=1) as res,
            tc.tile_pool(name="xa", bufs=1) as xa,
            tc.tile_pool(name="qa", bufs=1) as qa,
            tc.tile_pool(name="ta", bufs=1) as ta,
            tc.tile_pool(name="eb", bufs=1) as eb,
            tc.tile_pool(name="tb", bufs=1) as tb,
            tc.tile_pool(name="xc", bufs=1) as xc,
            tc.tile_pool(name="tco", bufs=1) as tco,
            tc.tile_pool(name="dram", bufs=1, space="DRAM") as dram,
            tc.tile_pool(name="psP", bufs=1, space="PSUM") as psP,
        ):
            # ---- resident tiles -------------------------------------------
            msk_t = res.tile([128, 4, 512], BF16)
            ones_m = res.tile([128, 128], BF16)
            kT_t = res.tile([128, NI, S], BF16)    # rope'd K^T per instance
            v_t = res.tile([128, B * 16, HD], BF16)  # V natural [s, dv]
            wo_t = res.tile([128, NKT, OSL], BF16)
            wq_t = res.tile([128, NKT, HD], BF16)
            wk_t = res.tile([128, NKT, HD], BF16)
            wv_t = res.tile([128, NKT, HD], BF16)
            cos_t = res.tile([128, S], BF16)
            sin_t = res.tile([128, S], BF16)

            # input DMAs, in first-use order
            xt_tiles = {}
            for sb in range(NSB):
                for b in range(B):
                    xt_tiles[(b, sb)] = xa.tile(
                        [128, NKT, SBW], BF16, tag="xt", bufs=2,
                        name=f"xt{b}{sb}")
            for i in range(NKT):
                nc.sync.dma_start(out=wq_t[:, i, :],
                                  in_=wqT[i * 128:(i + 1) * 128, :])
                nc.sync.dma_start(out=xt_tiles[(0, 0)][:, i, :],
                                  in_=xT0[i * 128:(i + 1) * 128, 0:SBW])
            for i in range(NKT):
                nc.sync.dma_start(out=wk_t[:, i, :],
                                  in_=wkT[i * 128:(i + 1) * 128, :])
            nc.sync.dma_start(out=cos_t[:], in_=cosb[:])
            nc.sync.dma_start(out=sin_t[:], in_=sinb[:])
            for i in range(NKT):
                nc.sync.dma_start(out=wv_t[:, i, :],
                                  in_=wvT[i * 128:(i + 1) * 128, :])
            for j in range(4):
                nc.sync.dma_start(out=msk_t[:, j, :], in_=msk[j])
            nc.vector.memset(ones_m[:], 1.0)

            def stage_a2a(qb, ao_c):
                """Issue AllToAll for chunk qb + the gather-side loads."""
                ag_in = dram.tile([NCORES * HD, 512], BF16, tag="agin",
                                  bufs=2, name=f"agin{qb}")
                for d in range(NCORES):
                    bb = d // 4
                    for j in range(HPC):
                        r0 = d * HD + j * 128
                        nc.sync.dma_start(
                            out=ag_in[r0:r0 + 128, :],
                            in_=ao_c[:, bb * HPC + j, :])
                ag_out = dram.tile([NCORES * HD, 512], BF16, tag="agout",
                                   bufs=2, name=f"agout{qb}")
                nc.gpsimd.collective_compute(
                    "AllToAll",
                    mybir.AluOpType.bypass,
                    ins=[ag_in.opt()],
                    outs=[ag_out.opt()],
                    replica_groups=GROUP,
                )
                agt_a = xc.tile([128, 8, 512], BF16, tag="agta", bufs=2,
                                name=f"agta{qb}")
                agt_b = xc.tile([128, 8, 512], BF16, tag="agtb", bufs=2,
                                name=f"agtb{qb}")
                for i in range(8):
                    nc.sync.dma_start(
                        out=agt_a[:, i, :],
                        in_=ag_out[i * 128:(i + 1) * 128, :])
                for i in range(8):
                    nc.sync.dma_start(
                        out=agt_b[:, i, :],
                        in_=ag_out[(i + 8) * 128:(i + 9) * 128, :])
                return (agt_a, agt_b)

            # ---- pipeline over the 4 sequence chunks (ascending),
            #      projC trailing two chunks behind -----------------------
            agt_map = {}
            for t in range(NSB):
                for b in range(B):
                    if (b, t) != (0, 0):
                        xt = xt_tiles[(b, t)]
                        for i in range(NKT):
                            nc.sync.dma_start(
                                out=xt[:, i, :],
                                in_=xTs[b][i * 128:(i + 1) * 128,
                                           t * SBW:(t + 1) * SBW])
                if t == 1:
                    for i in range(NKT):
                        nc.sync.dma_start(out=wo_t[:, i, :],
                                          in_=woT[i * 128:(i + 1) * 128, :])
                qTc = qa.tile([128, NI, SBW], BF16, tag="qT", bufs=2,
                              name=f"qT{t}")
                for b in range(B):
                    _proj_block(nc, b, t, xt_tiles[(b, t)], wq_t, wk_t, wv_t,
                                cos_t, sin_t, qTc, kT_t, v_t, ta, psP)
                ao_c = qa.tile([128, NI, SBW], BF16, tag="ao", bufs=2,
                               name=f"ao{t}")
                for hi in range(NI):
                    _att_block(nc, t, hi, qTc, kT_t, v_t, ao_c, msk_t,
                               ones_m, eb, tb, psP)
                agt_map[t] = stage_a2a(t, ao_c)
                if t >= 2:
                    _out_proj_block(nc, t - 2, agt_map[t - 2], wo_t, out,
                                    tco, psP)
            _out_proj_block(nc, NSB - 2, agt_map[NSB - 2], wo_t, out, tco,
                            psP)
            _out_proj_block(nc, NSB - 1, agt_map[NSB - 1], wo_t, out, tco,
                            psP)
    nc.compile()
    return nc


def _host_prep(x, wq, wk, wv, wo):
    """Build per-core input maps (host-side transposes + bf16 casts)."""
    bf = ml_dtypes.bfloat16
    # rope tables in the transposed [d, s] layout
    inv = 1.0 / (10000.0 ** (np.arange(0, D, 2, dtype=np.float64) / D))  # [64]
    ang = np.outer(np.arange(S, dtype=np.float64), inv)                  # [S, 64]
    cos = np.cos(ang).T        # [64, S]
    sin = np.sin(ang).T        # [64, S]
    cosb = np.repeat(cos, 2, axis=0).astype(np.float32)                  # [128, S]
    sinb = np.repeat(sin, 2, axis=0).astype(np.float32)
    sinb[0::2, :] *= -1.0      # even d rows: -sin ; odd rows: +sin

    ki = np.arange(128)[:, None]
    qj = np.arange(512)[None, :]
    msk_m = np.stack([(j * 128 + ki <= qj).astype(np.float32) for j in range(4)])

    xT_b = [np.ascontiguousarray(x[b].T).astype(bf) for b in range(B)]
    cosb, sinb = cosb.astype(bf), sinb.astype(bf)
    msk_m = msk_m.astype(bf)

    in_maps = []
    for c in range(NCORES):
        hrows = slice(c * HD, (c + 1) * HD)          # this core's 2 heads
        ocols = slice((c % 4) * OSL, (c % 4 + 1) * OSL)  # its output columns
        in_maps.append({
            "xT0": xT_b[0],
            "xT1": xT_b[1],
            "wqT": np.ascontiguousarray(wq[hrows].T).astype(bf),
            "wkT": np.ascontiguousarray(wk[hrows].T).astype(bf),
            "wvT": np.ascontiguousarray(wv[hrows].T).astype(bf),
            "woT": np.ascontiguousarray(wo[ocols, :].T).astype(bf),
            "cosb": cosb,
            "sinb": sinb,
            "msk": msk_m,
        })
    return in_maps


def kernel(x, wq, wk, wv, wo):
    global LAST_RESULT, _CACHED_NC
    if _CACHED_NC is None:
        _CACHED_NC = _build()
    nc = _CACHED_NC
    in_maps = _host_prep(x, wq, wk, wv, wo)
    res = run_bass_kernel_spmd(nc, in_maps, core_ids=list(range(NCORES)))
    LAST_RESULT = res
    out = np.empty((B, S, DIM), np.float32)
    for c in range(NCORES):
        bb = c // 4
        csl = slice((c % 4) * OSL, (c % 4 + 1) * OSL)
        out[bb, :, csl] = res.results[c]["out"]
    return out
